# revision 1
# baseline (speedup 1.0000x reference)
"""MoE routing kernel for Trainium2 (8 NeuronCores, SPMD data-parallel).

Computes, for x [4, 4096, 4096] f32, proto_k [64, 4096] f32, gate [64] f32:
    logits = relu(x @ proto_k.T / sqrt(4096) - gate)        # [B, S, 64]
    routing_weights, selected_experts = top_k(logits, k=8)  # [B, S, 8] each

Sharding: tokens (B*S = 16384) are split evenly across 8 cores (2048 each).
proto_k / gate are replicated. No collectives needed.

Numerics: the matmul runs as a 3-term fp16 hi/lo split (x = xh + xl,
proto = ph + pl, logits = xh@ph + xh@pl + xl@ph, dropping xl@pl ~ 2^-22).
The residuals are pre-scaled by 2^11 on the host so they stay in fp16's
normal range, accumulated in a second PSUM bank, and recombined as
hi + 2^-11 * lo on the DVE.  Validated: bit-noise-level agreement with the
fp32 reference (max logit perturbation ~4e-8, zero top-8 index flips),
while streaming the PE at fp16 rate (1 cycle/row, 3 passes) instead of
fp32's 4 cycles/row with serialized weight loads.

Per-core device program:
  - x shard is split/transposed on the host to xh/xl [4096, 2048] fp16 so
    every DMA is contiguous and the contraction dim rides SBUF partitions.
  - logits accumulate with experts on partitions: per 128-wide hidden chunk,
    3 matmuls into 2 PSUM banks ([64, 512] per 512-token group).
  - DVE recombines hi + 2^-11*lo; ScalarE applies relu(acc/64 - gate).
  - TensorE transposes [64, 128] tiles -> [128 tokens, 64 experts] PSUM.
  - DVE Max8/MaxIndex emit top-8 values (descending) + indices per token.
  - Outputs pack as [128, 16*8] tiles, unscrambled on the host.
"""

import numpy as np

HIDDEN = 4096
NUM_EXPERTS = 64
TOP_K = 8
N_CORES = 8
TOKENS = 4 * 4096
T_CORE = TOKENS // N_CORES          # 2048 tokens per core
N_CHUNK = HIDDEN // 128             # 32 contraction chunks
GROUPS_PER_PASS = 2                 # 512-token groups accumulated per pass
N_PASS = T_CORE // (512 * GROUPS_PER_PASS)
N_SUB = T_CORE // 128               # 16 output sub-tiles of 128 tokens
LO_SCALE = np.float32(2.0 ** 11)
LO_UNSCALE = 2.0 ** -11

_PROGRAM = None


def _split_multi_waits(nc):
    """walrus in this container rejects instructions carrying more sync waits
    than their ISA struct holds (setupSyncWait: 'Too many sync wait
    commands'); Drain takes one, S3_LW (matmul weight-load) ~two.  Normalize
    every instruction to a single wait by hoisting extras onto same-engine
    NOPs inserted immediately before the owner."""
    import bass_rust

    inserts = {}  # owner inst name -> list of wait-nop instructions
    for f in nc.m.functions:
        for bb in f.blocks:
            for inst in bb.instructions:
                si = inst.sync_info
                if si is None or len(si.on_wait) <= 1:
                    continue
                conds = list(si.on_wait)
                si.on_wait = conds[:1]
                eng = nc.engines[inst.engine]
                new_insts = []
                for w in conds[1:]:
                    nop = eng.nop(hint="split_wait")
                    nop.ins.sync_info = bass_rust.SyncInfo(
                        on_wait=[w], on_update=[]
                    )
                    new_insts.append(nop.ins)
                inserts[inst.name] = new_insts
    if not inserts:
        return
    # nop() appended the new instructions to whatever bb was current; strip
    # them from everywhere, then re-insert each right before its owner so
    # the engine observes every wait before executing the instruction.
    appended = {ni.name for nis in inserts.values() for ni in nis}
    for f in nc.m.functions:
        for bb in f.blocks:
            rebuilt = []
            changed = False
            for inst in bb.instructions:
                if inst.name in appended:
                    changed = True
                    continue
                if inst.name in inserts:
                    rebuilt.extend(inserts[inst.name])
                    changed = True
                rebuilt.append(inst)
            if changed:
                bb.instructions = rebuilt


def _build_program():
    import concourse.bass as bass
    import concourse.mybir as mybir
    import concourse.tile as tile

    f32 = mybir.dt.float32
    f16 = mybir.dt.float16
    u32 = mybir.dt.uint32
    E = NUM_EXPERTS

    nc = bass.Bass("TRN2", target_bir_lowering=False, debug=False)

    # xh and xl stacked: xhl[0] = hi, xhl[1] = lo (one DMA fetches both)
    xhl_d = nc.dram_tensor("xhl", [2, HIDDEN, T_CORE], f16, kind="ExternalInput")
    # proto hi|lo packed along expert columns: [:, 0:64] = ph, [:, 64:128] = pl
    phpl_d = nc.dram_tensor("phpl", [HIDDEN, 2 * E], f16, kind="ExternalInput")
    gate_neg = nc.dram_tensor("gate_neg", [E, 1], f32, kind="ExternalInput")
    w_out = nc.dram_tensor("w_out", [128, N_SUB * TOP_K], f32, kind="ExternalOutput")
    i_out = nc.dram_tensor("i_out", [128, N_SUB * TOP_K], u32, kind="ExternalOutput")

    ident_dram = nc.inline_tensor(np.eye(E, dtype=np.float32), name="ident64")

    with tile.TileContext(nc) as tc:
        with (
            tc.tile_pool(name="const", bufs=1) as const_pool,
            tc.tile_pool(name="xa", bufs=12) as x_pool,
            tc.tile_pool(name="acc", bufs=7, space="PSUM") as acc_pool,
            tc.tile_pool(name="tp", bufs=1, space="PSUM") as tp_pool,
            tc.tile_pool(name="lg", bufs=3) as lg_pool,
            tc.tile_pool(name="tk", bufs=3) as tk_pool,
            tc.tile_pool(name="outp", bufs=1) as out_pool,
        ):
            # --- constants ---
            # proto chunks land as [128, c, E]; per-chunk DMAs are contiguous
            # 32 KB and let the first matmuls start early.
            # weights ride the (otherwise idle) gpsimd SWDGE ring so neither
            # the x stream (sync ring) nor the epilogue traffic (scalar
            # ring) queues behind their 32 triggers.
            phpl_sb = const_pool.tile([128, N_CHUNK * 2 * E], f16)
            for c in range(N_CHUNK):
                nc.gpsimd.dma_start(
                    phpl_sb[:, c * 2 * E:(c + 1) * 2 * E],
                    phpl_d[c * 128:(c + 1) * 128, :],
                )
            gate_sb = const_pool.tile([E, 1], f32)
            nc.scalar.dma_start(gate_sb[:], gate_neg[:])
            ident_sb = const_pool.tile([E, E], f32)
            nc.scalar.dma_start(ident_sb[:], ident_dram[:])

            vals_sb = out_pool.tile([128, N_SUB * TOP_K], f32)
            idx_sb = out_pool.tile([128, N_SUB * TOP_K], u32)

            for p in range(N_PASS):
                tpp = GROUPS_PER_PASS * 512
                t0 = p * tpp
                # a = xh @ [ph|pl]: rows 0:64 main term, 64:128 lo (2^11)
                # b = xl @ [ph|pl]: rows 0:64 lo (2^11), 64:128 llo (2^22)
                a_accs = [
                    acc_pool.tile([128, 512], f32, name=f"a_p{p}g{g}", tag="acc")
                    for g in range(GROUPS_PER_PASS)
                ]
                b_accs = [
                    acc_pool.tile([128, 512], f32, name=f"b_p{p}g{g}", tag="acc")
                    for g in range(GROUPS_PER_PASS)
                ]
                for c in range(N_CHUNK):
                    # one HWDGE DMA per chunk fetches hi and lo halves;
                    # alternate between the two HWDGE rings (SP / ACT) so
                    # trigger issue is never the bottleneck
                    x_t = x_pool.tile([128, 2, tpp], f16, name="x_t", tag="xt")
                    src = (xhl_d[:, c * 128:(c + 1) * 128, t0:t0 + tpp]
                           .rearrange("s p t -> p s t"))
                    if p == 0 and c == 0:
                        # split the very first chunk by stream and group
                        # across both rings: the first matmul then waits on
                        # a 128 KB transfer instead of 512 KB
                        nc.sync.dma_start(x_t[:, 0, 0:512], src[:, 0, 0:512])
                        nc.scalar.dma_start(x_t[:, 1, 0:512], src[:, 1, 0:512])
                        nc.sync.dma_start(x_t[:, 0, 512:tpp], src[:, 0, 512:tpp])
                        nc.scalar.dma_start(x_t[:, 1, 512:tpp], src[:, 1, 512:tpp])
                    else:
                        ring = nc.sync if c % 2 == 0 else nc.scalar
                        ring.dma_start(x_t[:], src)
                    first, last = (c == 0), (c == N_CHUNK - 1)
                    pc = slice(c * 2 * E, (c + 1) * 2 * E)
                    # on the final chunk of the final pass, close the groups
                    # in reverse so the tail-critical epilogue starts while
                    # the other group's last matmuls still run
                    grange = (reversed(range(GROUPS_PER_PASS))
                              if (last and p == N_PASS - 1)
                              else range(GROUPS_PER_PASS))
                    for g in grange:
                        ts = slice(g * 512, (g + 1) * 512)
                        nc.tensor.matmul(
                            a_accs[g][:], phpl_sb[:, pc], x_t[:, 0, ts],
                            start=first, stop=last,
                        )
                        nc.tensor.matmul(
                            b_accs[g][:], phpl_sb[:, pc], x_t[:, 1, ts],
                            start=first, stop=last,
                        )
                erange = (list(reversed(range(GROUPS_PER_PASS)))
                          if p == N_PASS - 1 else list(range(GROUPS_PER_PASS)))
                for g in erange:
                    # comb = a[0:64] + 2^-11*(a[64:128] + b[0:64] + 2^-11*b[64:128])
                    # DVE reads at most one PSUM input per op, so `a` is
                    # staged through SBUF (which also releases its PSUM bank
                    # for the next pass early).  The reads of the [64:128]
                    # halves into 0:64-partition outputs are cross-partition
                    # APs — verified exact on hardware.
                    a_sb = lg_pool.tile([128, 512], f32, name="a_sb")
                    nc.vector.tensor_copy(a_sb[:], a_accs[g][:])
                    u = lg_pool.tile([E, 512], f32, name="u")
                    nc.vector.scalar_tensor_tensor(
                        u[:], b_accs[g][0:E, :], 1.0, a_sb[E:2 * E, :],
                        bass.mybir.AluOpType.mult, bass.mybir.AluOpType.add,
                    )
                    v = lg_pool.tile([E, 512], f32, name="v")
                    nc.vector.scalar_tensor_tensor(
                        v[:], b_accs[g][E:2 * E, :], LO_UNSCALE, u[:],
                        bass.mybir.AluOpType.mult, bass.mybir.AluOpType.add,
                    )
                    comb = lg_pool.tile([E, 512], f32, name="comb")
                    nc.vector.scalar_tensor_tensor(
                        comb[:], v[:], LO_UNSCALE, a_sb[0:E, :],
                        bass.mybir.AluOpType.mult, bass.mybir.AluOpType.add,
                    )
                    # relu(acc/64 - gate)  (ScalarE, SBUF -> SBUF)
                    logits = lg_pool.tile([E, 512], f32, name="logits")
                    nc.scalar.activation(
                        logits[:], comb[:],
                        bass.mybir.ActivationFunctionType.Relu,
                        bias=gate_sb[:], scale=1.0 / 64.0,
                    )
                    # transpose to [128 tokens, 64 experts] x 4 sub-tiles
                    tk_psum = tp_pool.tile([128, 4 * E], f32, name="tk_psum")
                    for j in range(4):
                        nc.tensor.transpose(
                            tk_psum[:, j * E:(j + 1) * E],
                            logits[:, j * 128:(j + 1) * 128],
                            ident_sb[:],
                        )
                    tk_sb = tk_pool.tile([128, 4 * E], f32, name="tk_sb")
                    nc.vector.tensor_copy(tk_sb[:], tk_psum[:])
                    gg = p * GROUPS_PER_PASS + g
                    for j in range(4):
                        s = gg * 4 + j
                        nc.vector.max(
                            vals_sb[:, s * TOP_K:(s + 1) * TOP_K],
                            tk_sb[:, j * E:(j + 1) * E],
                        )
                        nc.vector.max_index(
                            idx_sb[:, s * TOP_K:(s + 1) * TOP_K],
                            vals_sb[:, s * TOP_K:(s + 1) * TOP_K],
                            tk_sb[:, j * E:(j + 1) * E],
                        )
                # flush this pass's outputs so only the last pass's epilogue
                # sits in the kernel tail
                os_ = slice(p * GROUPS_PER_PASS * 4 * TOP_K,
                            (p + 1) * GROUPS_PER_PASS * 4 * TOP_K)
                nc.scalar.dma_start(w_out[:, os_], vals_sb[:, os_])
                nc.scalar.dma_start(i_out[:, os_], idx_sb[:, os_])

    _split_multi_waits(nc)
    return nc


def _get_program():
    global _PROGRAM
    if _PROGRAM is None:
        _PROGRAM = _build_program()
    return _PROGRAM


def _make_in_maps(x, proto_k, gate):
    xf = np.ascontiguousarray(x, dtype=np.float32).reshape(TOKENS, HIDDEN)
    proto = np.asarray(proto_k, dtype=np.float32)
    ph = proto.astype(np.float16)
    pl = ((proto - ph.astype(np.float32)) * LO_SCALE).astype(np.float16)
    phpl = np.concatenate([ph.T, pl.T], axis=1)           # [4096, 128] f16
    gate_neg = np.ascontiguousarray(
        -np.asarray(gate, dtype=np.float32).reshape(NUM_EXPERTS, 1)
    )
    in_maps = []
    for c in range(N_CORES):
        shard_t = xf[c * T_CORE:(c + 1) * T_CORE].T       # [4096, 2048] view
        xhl = np.empty((2, HIDDEN, T_CORE), np.float16)
        xhl[0] = shard_t
        xhl[1] = (shard_t - xhl[0].astype(np.float32)) * LO_SCALE
        in_maps.append(
            {"xhl": xhl, "phpl": phpl, "gate_neg": gate_neg}
        )
    return in_maps


def _gather(results):
    w = np.empty((TOKENS, TOP_K), np.float32)
    idx = np.empty((TOKENS, TOP_K), np.int32)
    for c in range(N_CORES):
        wo = results[c]["w_out"]                          # [128, 16*8]
        io = results[c]["i_out"].view(np.int32)
        w[c * T_CORE:(c + 1) * T_CORE] = (
            wo.reshape(128, N_SUB, TOP_K).transpose(1, 0, 2).reshape(T_CORE, TOP_K)
        )
        idx[c * T_CORE:(c + 1) * T_CORE] = (
            io.reshape(128, N_SUB, TOP_K).transpose(1, 0, 2).reshape(T_CORE, TOP_K)
        )
    return w.reshape(4, 4096, TOP_K), idx.reshape(4, 4096, TOP_K)


def run_sharded(in_maps, trace=False, trace_cores=None):
    from concourse.bass_utils import run_bass_kernel_spmd

    nc = _get_program()
    return run_bass_kernel_spmd(
        nc,
        in_maps,
        core_ids=list(range(N_CORES)),
        trace=trace,
        trace_cores=trace_cores,
    )


def kernel(x, proto_k, gate):
    in_maps = _make_in_maps(x, proto_k, gate)
    res = run_sharded(in_maps, trace=False)
    return _gather(res.results)



# revision 8
# speedup vs baseline: 1.0475x; 1.0475x over previous
"""MoE routing kernel for Trainium2 (8 NeuronCores, SPMD data-parallel).

Computes, for x [4, 4096, 4096] f32, proto_k [64, 4096] f32, gate [64] f32:
    logits = relu(x @ proto_k.T / sqrt(4096) - gate)        # [B, S, 64]
    routing_weights, selected_experts = top_k(logits, k=8)  # [B, S, 8] each

Sharding: tokens (B*S = 16384) are split evenly across 8 cores (2048 each).
proto_k / gate are replicated. No collectives needed.

Numerics: the matmul runs as a 4-term fp16 hi/lo split (x = xh + 2^-11 xl,
proto = ph + 2^-11 pl, residuals pre-scaled by 2^11 on the host so they stay
in fp16's normal range). logits = xh@ph + 2^-11 (xh@pl + xl@ph) + 2^-22
xl@pl, recombined on the DVE.  Bit-noise-level agreement with the fp32
reference (max logit perturbation ~5e-8, zero top-8 index flips) while
streaming the PE at fp16 rate.

The kernel is HBM-bandwidth bound (32 MB of x per core).  Device program:
  - x is laid out on the host as per-pass [chunk][row 128][hi|lo][token]
    fp16 blocks so every DMA reads *sequential* DRAM with 4 KB(+) contiguous
    per-partition lines; transfers are ~1.5 MB (2 chunks) on the two HWDGE
    rings (sync / scalar) running concurrently.
  - Tokens are processed in UNEVEN passes of [768, 768, 384, 128] with
    512/256-token PSUM accumulation groups.  Every pass's epilogue
    (hi/lo recombine -> relu-gate -> transpose -> top-8) overlaps the next
    pass's DMA stream; only the tiny 128-token final pass's epilogue is
    exposed after the last HBM byte.
  - Epilogue work is spread across engines: ACT (PSUM copies, relu), DVE
    (combines, Max8/MaxIndex8), PE (transposes), so the DVE never becomes
    the tail bottleneck.
  - Weights/gate/identity and the mid-run output flushes ride the gpsimd
    SWDGE ring; the final 128-token flush uses the then-idle HWDGE rings.
"""

import numpy as np

HIDDEN = 4096
NUM_EXPERTS = 64
TOP_K = 8
N_CORES = 8
TOKENS = 4 * 4096
T_CORE = TOKENS // N_CORES          # 2048 tokens per core
N_CHUNK = HIDDEN // 128             # 32 contraction chunks
N_SUB = T_CORE // 128               # 16 output sub-tiles of 128 tokens
LO_SCALE = np.float32(2.0 ** 11)
LO_UNSCALE = 2.0 ** -11

# uneven pass sizes: the last pass is tiny so the only epilogue that cannot
# overlap the DMA stream is as short as possible
PASS_TOK = [768, 768, 384, 128]
PASS_GROUPS = [[512, 256], [512, 256], [384], [128]]
# chunks per x dma_start per pass (transfer sizes ~1.5/1.5/0.75/0.5 MB)
PASS_CHDMA = [2, 2, 2, 4]
assert sum(PASS_TOK) == T_CORE

_PROGRAM = None


def _split_multi_waits(nc):
    """walrus in this container rejects instructions carrying more sync waits
    than their ISA struct holds (setupSyncWait: 'Too many sync wait
    commands'); Drain takes one, S3_LW (matmul weight-load) ~two.  Normalize
    every instruction to a single wait by hoisting extras onto same-engine
    NOPs inserted immediately before the owner."""
    import bass_rust

    inserts = {}  # owner inst name -> list of wait-nop instructions
    for f in nc.m.functions:
        for bb in f.blocks:
            for inst in bb.instructions:
                si = inst.sync_info
                if si is None or len(si.on_wait) <= 1:
                    continue
                conds = list(si.on_wait)
                si.on_wait = conds[:1]
                eng = nc.engines[inst.engine]
                new_insts = []
                for w in conds[1:]:
                    nop = eng.nop(hint="split_wait")
                    nop.ins.sync_info = bass_rust.SyncInfo(
                        on_wait=[w], on_update=[]
                    )
                    new_insts.append(nop.ins)
                inserts[inst.name] = new_insts
    if not inserts:
        return
    # nop() appended the new instructions to whatever bb was current; strip
    # them from everywhere, then re-insert each right before its owner so
    # the engine observes every wait before executing the instruction.
    appended = {ni.name for nis in inserts.values() for ni in nis}
    for f in nc.m.functions:
        for bb in f.blocks:
            rebuilt = []
            changed = False
            for inst in bb.instructions:
                if inst.name in appended:
                    changed = True
                    continue
                if inst.name in inserts:
                    rebuilt.extend(inserts[inst.name])
                    changed = True
                rebuilt.append(inst)
            if changed:
                bb.instructions = rebuilt


def _build_program():
    import concourse.bass as bass
    import concourse.mybir as mybir
    import concourse.tile as tile

    f32 = mybir.dt.float32
    f16 = mybir.dt.float16
    u32 = mybir.dt.uint32
    E = NUM_EXPERTS
    N_PASS = len(PASS_TOK)

    nc = bass.Bass("TRN2", target_bir_lowering=False, debug=False)

    # per-pass x blocks, host-reordered to [chunk][row in chunk][hi|lo][tok]
    # fp16: fully sequential DRAM, 4KB+ contiguous per-partition lines.
    xp_d = [
        nc.dram_tensor(f"xp{p}", [N_CHUNK, 128, 2, PASS_TOK[p]], f16,
                       kind="ExternalInput")
        for p in range(N_PASS)
    ]
    # proto hi|lo, host-reordered to [row in chunk][chunk][2E]
    phpl_d = nc.dram_tensor("phpl", [128, N_CHUNK, 2 * E], f16,
                            kind="ExternalInput")
    gate_neg = nc.dram_tensor("gate_neg", [E, 1], f32, kind="ExternalInput")
    w_out = nc.dram_tensor("w_out", [128, N_SUB * TOP_K], f32, kind="ExternalOutput")
    i_out = nc.dram_tensor("i_out", [128, N_SUB * TOP_K], u32, kind="ExternalOutput")

    ident_dram = nc.inline_tensor(np.eye(E, dtype=np.float32), name="ident64")

    with tile.TileContext(nc) as tc:
        with (
            tc.tile_pool(name="const", bufs=1) as const_pool,
            tc.tile_pool(name="xa", bufs=8) as x_pool,
            tc.tile_pool(name="acc", bufs=6, space="PSUM") as acc_pool,
            tc.tile_pool(name="tp", bufs=2, space="PSUM") as tp_pool,
            tc.tile_pool(name="lg", bufs=3) as lg_pool,
            tc.tile_pool(name="outp", bufs=1) as out_pool,
        ):
            # --- constants (gpsimd SWDGE ring: off the x-stream rings) ---
            phpl_sb = const_pool.tile([128, N_CHUNK, 2 * E], f16)
            # chunk 0 separately so the first matmul's weights land early
            nc.gpsimd.dma_start(phpl_sb[:, 0, :], phpl_d[:, 0, :])
            nc.gpsimd.dma_start(phpl_sb[:, 1:, :], phpl_d[:, 1:, :])
            gate_sb = const_pool.tile([E, 1], f32)
            nc.gpsimd.dma_start(gate_sb[:], gate_neg[:])
            ident_sb = const_pool.tile([E, E], f32)
            nc.gpsimd.dma_start(ident_sb[:], ident_dram[:])

            vals_sb = out_pool.tile([128, N_SUB * TOP_K], f32)
            idx_sb = out_pool.tile([128, N_SUB * TOP_K], u32)

            rings = [nc.sync, nc.scalar]
            ring_state = [0]

            def next_ring():
                r = rings[ring_state[0] & 1]
                ring_state[0] += 1
                return r

            sub_base = 0  # running 128-token output subtile index
            for p in range(N_PASS):
                T = PASS_TOK[p]
                groups = PASS_GROUPS[p]
                goff = np.cumsum([0] + groups)[:-1]
                a_accs = [
                    acc_pool.tile([128, w], f32, name=f"a_p{p}g{g}", tag="acc")
                    for g, w in enumerate(groups)
                ]
                b_accs = [
                    acc_pool.tile([128, w], f32, name=f"b_p{p}g{g}", tag="acc")
                    for g, w in enumerate(groups)
                ]

                # ---- x stream for this pass ----
                c = 0
                slot_of = {}
                last = N_PASS - 1
                while c < N_CHUNK:
                    if p == 0 and c == 0:
                        # split the very first chunk by stream and half so
                        # the first matmul waits on ~96 KB, not 1.5 MB
                        x_t = x_pool.tile([128, 1, 2, T], f16, name="x_h", tag="xt")
                        src = xp_d[p][0:1].rearrange("c r s t -> r c s t")
                        h = groups[0]
                        nc.sync.dma_start(x_t[:, 0, 0, 0:h], src[:, 0, 0, 0:h])
                        nc.scalar.dma_start(x_t[:, 0, 1, 0:h], src[:, 0, 1, 0:h])
                        nc.sync.dma_start(x_t[:, 0, 0, h:], src[:, 0, 0, h:])
                        nc.scalar.dma_start(x_t[:, 0, 1, h:], src[:, 0, 1, h:])
                        slot_of[0] = (x_t, 0)
                        c += 1
                    elif p == last and c == N_CHUNK - 2:
                        # final two chunks: one small transfer per ring so
                        # the last bytes land soon and rings drain together
                        x_t = x_pool.tile([128, 2, 2, T], f16, name="x_z", tag="xt")
                        src = xp_d[p][c:c + 2].rearrange("c r s t -> r c s t")
                        next_ring().dma_start(x_t[:, 0], src[:, 0])
                        next_ring().dma_start(x_t[:, 1], src[:, 1])
                        slot_of[c] = (x_t, 0)
                        slot_of[c + 1] = (x_t, 1)
                        c += 2
                    else:
                        n = min(PASS_CHDMA[p], N_CHUNK - c)
                        if p == last:
                            n = min(n, N_CHUNK - 2 - c)
                        x_t = x_pool.tile([128, n, 2, T], f16, name="x_t", tag="xt")
                        src = xp_d[p][c:c + n].rearrange("c r s t -> r c s t")
                        next_ring().dma_start(x_t[:], src)
                        for j in range(n):
                            slot_of[c + j] = (x_t, j)
                        c += n

                # ---- accumulation matmuls ----
                for c in range(N_CHUNK):
                    x_t, j = slot_of[c]
                    first, lastc = (c == 0), (c == N_CHUNK - 1)
                    grange = (
                        reversed(range(len(groups)))
                        if (lastc and p == N_PASS - 1)
                        else range(len(groups))
                    )
                    for g in grange:
                        ts = slice(goff[g], goff[g] + groups[g])
                        nc.tensor.matmul(
                            a_accs[g][:], phpl_sb[:, c, :], x_t[:, j, 0, ts],
                            start=first, stop=lastc,
                        )
                        nc.tensor.matmul(
                            b_accs[g][:], phpl_sb[:, c, :], x_t[:, j, 1, ts],
                            start=first, stop=lastc,
                        )

                # ---- epilogue per group ----
                # comb = a_hi + 2^-11 (a_lo + b_hi) + 2^-22 b_lo.  ACT does
                # the PSUM->SBUF copies and relu; DVE does the recombines and
                # Max8/MaxIndex8; PE transposes.  For every pass but the
                # last this overlaps the next pass's DMA stream.
                for g in reversed(range(len(groups))) if p == N_PASS - 1 \
                        else range(len(groups)):
                    W = groups[g]
                    nsub = W // 128
                    a_sb = lg_pool.tile([128, W], f32, name="a_sb", tag="a")
                    nc.scalar.activation(
                        a_sb[:], a_accs[g][:],
                        bass.mybir.ActivationFunctionType.Copy,
                    )
                    u = lg_pool.tile([E, W], f32, name="u", tag="u")
                    nc.vector.scalar_tensor_tensor(
                        u[:], b_accs[g][0:E, :], 1.0, a_sb[E:2 * E, :],
                        bass.mybir.AluOpType.mult, bass.mybir.AluOpType.add,
                    )
                    v = lg_pool.tile([E, W], f32, name="v", tag="v")
                    nc.vector.scalar_tensor_tensor(
                        v[:], b_accs[g][E:2 * E, :], LO_UNSCALE, u[:],
                        bass.mybir.AluOpType.mult, bass.mybir.AluOpType.add,
                    )
                    comb = lg_pool.tile([E, W], f32, name="comb", tag="c")
                    nc.vector.scalar_tensor_tensor(
                        comb[:], v[:], LO_UNSCALE, a_sb[0:E, :],
                        bass.mybir.AluOpType.mult, bass.mybir.AluOpType.add,
                    )
                    logits = lg_pool.tile([E, W], f32, name="logits", tag="l")
                    nc.scalar.activation(
                        logits[:], comb[:],
                        bass.mybir.ActivationFunctionType.Relu,
                        bias=gate_sb[:], scale=1.0 / 64.0,
                    )
                    tk_psum = tp_pool.tile([128, 4 * E], f32, name="tk_psum")
                    for jj in range(nsub):
                        nc.tensor.transpose(
                            tk_psum[:, jj * E:(jj + 1) * E],
                            logits[:, jj * 128:(jj + 1) * 128],
                            ident_sb[:],
                        )
                    tk_sb = lg_pool.tile([128, 4 * E], f32, name="tk_sb", tag="t")
                    nc.scalar.activation(
                        tk_sb[:, 0:nsub * E], tk_psum[:, 0:nsub * E],
                        bass.mybir.ActivationFunctionType.Copy,
                    )
                    gsub = sub_base + goff[g] // 128
                    for jj in range(nsub):
                        s = gsub + jj
                        nc.vector.max(
                            vals_sb[:, s * TOP_K:(s + 1) * TOP_K],
                            tk_sb[:, jj * E:(jj + 1) * E],
                        )
                        nc.vector.max_index(
                            idx_sb[:, s * TOP_K:(s + 1) * TOP_K],
                            vals_sb[:, s * TOP_K:(s + 1) * TOP_K],
                            tk_sb[:, jj * E:(jj + 1) * E],
                        )
                # flush this pass's outputs.  Mid-run flushes ride the idle
                # gpsimd ring (the HWDGE rings must keep streaming x and an
                # in-order trigger would stall them); the final tiny flush
                # uses the by-then-idle HWDGE rings.
                os_ = slice(sub_base * TOP_K, (sub_base + T // 128) * TOP_K)
                if p < N_PASS - 1:
                    nc.gpsimd.dma_start(w_out[:, os_], vals_sb[:, os_])
                    nc.gpsimd.dma_start(i_out[:, os_], idx_sb[:, os_])
                else:
                    nc.sync.dma_start(w_out[:, os_], vals_sb[:, os_])
                    nc.scalar.dma_start(i_out[:, os_], idx_sb[:, os_])
                sub_base += T // 128

    _split_multi_waits(nc)
    return nc


def _get_program():
    global _PROGRAM
    if _PROGRAM is None:
        _PROGRAM = _build_program()
    return _PROGRAM


def _make_in_maps(x, proto_k, gate):
    xf = np.ascontiguousarray(x, dtype=np.float32).reshape(TOKENS, HIDDEN)
    proto = np.asarray(proto_k, dtype=np.float32)
    ph = proto.astype(np.float16)
    pl = ((proto - ph.astype(np.float32)) * LO_SCALE).astype(np.float16)
    phpl = np.concatenate([ph.T, pl.T], axis=1)           # [4096, 128] f16
    # [row in chunk][chunk][2E]
    phpl_r = np.ascontiguousarray(
        phpl.reshape(N_CHUNK, 128, 2 * NUM_EXPERTS).transpose(1, 0, 2)
    )
    gate_neg = np.ascontiguousarray(
        -np.asarray(gate, dtype=np.float32).reshape(NUM_EXPERTS, 1)
    )
    toff = np.cumsum([0] + PASS_TOK)
    in_maps = []
    for cid in range(N_CORES):
        shard = xf[cid * T_CORE:(cid + 1) * T_CORE]       # [2048, 4096]
        hi = shard.astype(np.float16)
        lo = ((shard - hi.astype(np.float32)) * LO_SCALE).astype(np.float16)
        hi_t = hi.T.reshape(N_CHUNK, 128, T_CORE)         # [chunk, row, tok]
        lo_t = lo.T.reshape(N_CHUNK, 128, T_CORE)
        m = {"phpl": phpl_r, "gate_neg": gate_neg}
        for p, T in enumerate(PASS_TOK):
            xp = np.empty((N_CHUNK, 128, 2, T), np.float16)
            xp[:, :, 0, :] = hi_t[:, :, toff[p]:toff[p + 1]]
            xp[:, :, 1, :] = lo_t[:, :, toff[p]:toff[p + 1]]
            m[f"xp{p}"] = xp
        in_maps.append(m)
    return in_maps


def _gather(results):
    w = np.empty((TOKENS, TOP_K), np.float32)
    idx = np.empty((TOKENS, TOP_K), np.int32)
    for c in range(N_CORES):
        wo = results[c]["w_out"]                          # [128, 16*8]
        io = results[c]["i_out"].view(np.int32)
        w[c * T_CORE:(c + 1) * T_CORE] = (
            wo.reshape(128, N_SUB, TOP_K).transpose(1, 0, 2).reshape(T_CORE, TOP_K)
        )
        idx[c * T_CORE:(c + 1) * T_CORE] = (
            io.reshape(128, N_SUB, TOP_K).transpose(1, 0, 2).reshape(T_CORE, TOP_K)
        )
    return w.reshape(4, 4096, TOP_K), idx.reshape(4, 4096, TOP_K)


def run_sharded(in_maps, trace=False, trace_cores=None):
    from concourse.bass_utils import run_bass_kernel_spmd

    nc = _get_program()
    return run_bass_kernel_spmd(
        nc,
        in_maps,
        core_ids=list(range(N_CORES)),
        trace=trace,
        trace_cores=trace_cores,
    )


def kernel(x, proto_k, gate):
    in_maps = _make_in_maps(x, proto_k, gate)
    res = run_sharded(in_maps, trace=False)
    return _gather(res.results)


# revision 13
# speedup vs baseline: 1.0991x; 1.0493x over previous
"""MoE routing kernel for Trainium2 (8 NeuronCores, SPMD data-parallel).

Computes, for x [4, 4096, 4096] f32, proto_k [64, 4096] f32, gate [64] f32:
    logits = relu(x @ proto_k.T / sqrt(4096) - gate)        # [B, S, 64]
    routing_weights, selected_experts = top_k(logits, k=8)  # [B, S, 8] each

Sharding: tokens (B*S = 16384) are split evenly across 8 cores (2048 each).
proto_k / gate are replicated. No collectives needed.

Numerics: the matmul runs as a 4-term fp16 hi/lo split (x = xh + 2^-11 xl,
proto = ph + 2^-11 pl, residuals pre-scaled by 2^11 on the host so they stay
in fp16's normal range). logits = xh@ph + 2^-11 (xh@pl + xl@ph) + 2^-22
xl@pl, recombined on the DVE.  Bit-noise-level agreement with the fp32
reference (max logit perturbation ~5e-8, zero top-8 index flips) while
streaming the PE at fp16 rate.

The kernel is HBM-bandwidth bound (32 MB of x per core).  Device program:
  - x is laid out on the host as per-pass [chunk][row 128][hi|lo][token]
    fp16 blocks so every DMA reads *sequential* DRAM with 4 KB(+) contiguous
    per-partition lines; transfers are ~1.5 MB (2 chunks) on the two HWDGE
    rings (sync / scalar) running concurrently.
  - Tokens are processed in UNEVEN passes of [768, 768, 384, 128] with
    512/256-token PSUM accumulation groups.  Every pass's epilogue
    (hi/lo recombine -> relu-gate -> transpose -> top-8) overlaps the next
    pass's DMA stream; only the tiny 128-token final pass's epilogue is
    exposed after the last HBM byte.
  - Epilogue work is spread across engines: ACT (PSUM copies, relu), DVE
    (combines, Max8/MaxIndex8), PE (transposes), so the DVE never becomes
    the tail bottleneck.
  - Weights/gate/identity and the mid-run output flushes ride the gpsimd
    SWDGE ring; the final 128-token flush uses the then-idle HWDGE rings.
"""

import numpy as np

HIDDEN = 4096
NUM_EXPERTS = 64
TOP_K = 8
N_CORES = 8
TOKENS = 4 * 4096
T_CORE = TOKENS // N_CORES          # 2048 tokens per core
N_CHUNK = HIDDEN // 128             # 32 contraction chunks
N_SUB = T_CORE // 128               # 16 output sub-tiles of 128 tokens
LO_SCALE = np.float32(2.0 ** 11)
LO_UNSCALE = 2.0 ** -11

# uneven pass sizes: the last pass is tiny so the only epilogue that cannot
# overlap the DMA stream is as short as possible
PASS_TOK = [768, 768, 384, 128]
PASS_GROUPS = [[512, 256], [512, 256], [384], [128]]
# chunks per x dma_start per pass (transfer sizes ~1.5/1.5/0.75/0.5 MB)
PASS_CHDMA = [2, 2, 2, 4]
assert sum(PASS_TOK) == T_CORE

_PROGRAM = None


def _split_multi_waits(nc):
    """walrus in this container rejects instructions carrying more sync waits
    than their ISA struct holds (setupSyncWait: 'Too many sync wait
    commands'); Drain takes one, S3_LW (matmul weight-load) ~two.  Normalize
    every instruction to a single wait by hoisting extras onto same-engine
    NOPs inserted immediately before the owner."""
    import bass_rust

    inserts = {}  # owner inst name -> list of wait-nop instructions
    for f in nc.m.functions:
        for bb in f.blocks:
            for inst in bb.instructions:
                si = inst.sync_info
                if si is None or len(si.on_wait) <= 1:
                    continue
                conds = list(si.on_wait)
                si.on_wait = conds[:1]
                eng = nc.engines[inst.engine]
                new_insts = []
                for w in conds[1:]:
                    nop = eng.nop(hint="split_wait")
                    nop.ins.sync_info = bass_rust.SyncInfo(
                        on_wait=[w], on_update=[]
                    )
                    new_insts.append(nop.ins)
                inserts[inst.name] = new_insts
    if not inserts:
        return
    # nop() appended the new instructions to whatever bb was current; strip
    # them from everywhere, then re-insert each right before its owner so
    # the engine observes every wait before executing the instruction.
    appended = {ni.name for nis in inserts.values() for ni in nis}
    for f in nc.m.functions:
        for bb in f.blocks:
            rebuilt = []
            changed = False
            for inst in bb.instructions:
                if inst.name in appended:
                    changed = True
                    continue
                if inst.name in inserts:
                    rebuilt.extend(inserts[inst.name])
                    changed = True
                rebuilt.append(inst)
            if changed:
                bb.instructions = rebuilt


def _build_program():
    import concourse.bass as bass
    import concourse.mybir as mybir
    import concourse.tile as tile

    f32 = mybir.dt.float32
    f16 = mybir.dt.float16
    u32 = mybir.dt.uint32
    E = NUM_EXPERTS
    N_PASS = len(PASS_TOK)

    nc = bass.Bass("TRN2", target_bir_lowering=False, debug=False)

    # per-pass x blocks, host-reordered to [chunk][row in chunk][hi|lo][tok]
    # fp16: fully sequential DRAM, 4KB+ contiguous per-partition lines.
    xp_d = [
        nc.dram_tensor(f"xp{p}", [N_CHUNK, 128, 2, PASS_TOK[p]], f16,
                       kind="ExternalInput")
        for p in range(N_PASS)
    ]
    # proto hi|lo, host-reordered to [row in chunk][chunk][2E]
    phpl_d = nc.dram_tensor("phpl", [128, N_CHUNK, 2 * E], f16,
                            kind="ExternalInput")
    gate_row = nc.dram_tensor("gate_row", [1, E], f32, kind="ExternalInput")
    w_out = nc.dram_tensor("w_out", [128, N_SUB * TOP_K], f32, kind="ExternalOutput")
    i_out = nc.dram_tensor("i_out", [128, N_SUB * TOP_K], u32, kind="ExternalOutput")

    # scaled identities: the "transpose" matmuls fold the 1/sqrt(4096)
    # logit scale and the 2^-11 lo-term unscale into the PE accumulation,
    # and a rank-1 ones x (-gate) matmul folds the gate bias, so the DVE
    # epilogue is just two recombine ops + top-8.
    identA_dram = nc.inline_tensor(
        np.eye(E, dtype=np.float32) / 64.0, name="identA"
    )
    identB_dram = nc.inline_tensor(
        np.eye(E, dtype=np.float32) * (2.0 ** -17), name="identB"
    )
    ones_dram = nc.inline_tensor(np.ones((1, 128), dtype=np.float32), name="ones1")

    with tile.TileContext(nc) as tc:
        with (
            tc.tile_pool(name="const", bufs=1) as const_pool,
            tc.tile_pool(name="xa", bufs=8) as x_pool,
            tc.tile_pool(name="acc", bufs=6, space="PSUM") as acc_pool,
            tc.tile_pool(name="tp", bufs=2, space="PSUM") as tp_pool,
            tc.tile_pool(name="lg", bufs=3) as lg_pool,
            tc.tile_pool(name="outp", bufs=1) as out_pool,
        ):
            # --- constants (gpsimd SWDGE ring: off the x-stream rings) ---
            phpl_sb = const_pool.tile([128, N_CHUNK, 2 * E], f16)
            # chunk 0 separately so the first matmul's weights land early
            nc.gpsimd.dma_start(phpl_sb[:, 0, :], phpl_d[:, 0, :])
            nc.gpsimd.dma_start(phpl_sb[:, 1:, :], phpl_d[:, 1:, :])
            gate_sb = const_pool.tile([1, E], f32)
            nc.gpsimd.dma_start(gate_sb[:], gate_row[:])
            identA_sb = const_pool.tile([E, E], f32)
            nc.gpsimd.dma_start(identA_sb[:], identA_dram[:])
            identB_sb = const_pool.tile([E, E], f32)
            nc.gpsimd.dma_start(identB_sb[:], identB_dram[:])
            ones_sb = const_pool.tile([1, 128], f32)
            nc.gpsimd.dma_start(ones_sb[:], ones_dram[:])

            vals_sb = out_pool.tile([128, N_SUB * TOP_K], f32)
            idx_sb = out_pool.tile([128, N_SUB * TOP_K], u32)

            rings = [nc.sync, nc.scalar]
            ring_state = [0]

            def next_ring():
                r = rings[ring_state[0] & 1]
                ring_state[0] += 1
                return r

            sub_base = 0  # running 128-token output subtile index
            for p in range(N_PASS):
                T = PASS_TOK[p]
                groups = PASS_GROUPS[p]
                goff = np.cumsum([0] + groups)[:-1]
                a_accs = [
                    acc_pool.tile([128, w], f32, name=f"a_p{p}g{g}", tag="acc")
                    for g, w in enumerate(groups)
                ]
                b_accs = [
                    acc_pool.tile([128, w], f32, name=f"b_p{p}g{g}", tag="acc")
                    for g, w in enumerate(groups)
                ]

                # ---- x stream for this pass ----
                c = 0
                slot_of = {}
                last = N_PASS - 1
                while c < N_CHUNK:
                    if p == 0 and c == 0:
                        # split the very first chunk by stream and half so
                        # the first matmul waits on ~96 KB, not 1.5 MB
                        x_t = x_pool.tile([128, 1, 2, T], f16, name="x_h", tag="xt")
                        src = xp_d[p][0:1].rearrange("c r s t -> r c s t")
                        h = groups[0]
                        nc.sync.dma_start(x_t[:, 0, 0, 0:h], src[:, 0, 0, 0:h])
                        nc.scalar.dma_start(x_t[:, 0, 1, 0:h], src[:, 0, 1, 0:h])
                        nc.sync.dma_start(x_t[:, 0, 0, h:], src[:, 0, 0, h:])
                        nc.scalar.dma_start(x_t[:, 0, 1, h:], src[:, 0, 1, h:])
                        slot_of[0] = (x_t, 0)
                        c += 1
                    elif p == last and c == N_CHUNK - 2:
                        # final two chunks: one small transfer per ring so
                        # the last bytes land soon and rings drain together
                        x_t = x_pool.tile([128, 2, 2, T], f16, name="x_z", tag="xt")
                        src = xp_d[p][c:c + 2].rearrange("c r s t -> r c s t")
                        next_ring().dma_start(x_t[:, 0], src[:, 0])
                        next_ring().dma_start(x_t[:, 1], src[:, 1])
                        slot_of[c] = (x_t, 0)
                        slot_of[c + 1] = (x_t, 1)
                        c += 2
                    else:
                        n = min(PASS_CHDMA[p], N_CHUNK - c)
                        if p == last:
                            n = min(n, N_CHUNK - 2 - c)
                        x_t = x_pool.tile([128, n, 2, T], f16, name="x_t", tag="xt")
                        src = xp_d[p][c:c + n].rearrange("c r s t -> r c s t")
                        next_ring().dma_start(x_t[:], src)
                        for j in range(n):
                            slot_of[c + j] = (x_t, j)
                        c += n

                # ---- accumulation matmuls ----
                for c in range(N_CHUNK):
                    x_t, j = slot_of[c]
                    first, lastc = (c == 0), (c == N_CHUNK - 1)
                    grange = (
                        reversed(range(len(groups)))
                        if (lastc and p == N_PASS - 1)
                        else range(len(groups))
                    )
                    for g in grange:
                        ts = slice(goff[g], goff[g] + groups[g])
                        nc.tensor.matmul(
                            a_accs[g][:], phpl_sb[:, c, :], x_t[:, j, 0, ts],
                            start=first, stop=lastc,
                        )
                        nc.tensor.matmul(
                            b_accs[g][:], phpl_sb[:, c, :], x_t[:, j, 1, ts],
                            start=first, stop=lastc,
                        )

                # ---- epilogue per group ----
                # tk = [a_hi + 2^-11 (a_lo + b_hi) + 2^-22 b_lo]/64 - gate,
                # per 128-token subtile, transposed to [token, expert].  The
                # DVE does a PSUM->SBUF stage of `a` plus two recombines; the
                # PE's scaled-identity matmuls fold the transpose, the /64,
                # the 2^-11 unscale and the -gate bias; Max8/MaxIndex8 run
                # pre-relu (relu is monotone, so top-8 order is unchanged)
                # and the relu clamp lands on the [*, 8] outputs at pass end.
                # No epilogue op ever runs on the sync/scalar engines, whose
                # strict-FIFO queues must keep issuing x-stream DMA triggers.
                for g in reversed(range(len(groups))) if p == N_PASS - 1 \
                        else range(len(groups)):
                    W = groups[g]
                    nsub = W // 128
                    a_sb = lg_pool.tile([128, W], f32, name="a_sb", tag="a")
                    nc.vector.tensor_copy(a_sb[:], a_accs[g][:])
                    u = lg_pool.tile([E, W], f32, name="u", tag="u")
                    nc.vector.scalar_tensor_tensor(
                        u[:], b_accs[g][0:E, :], 1.0, a_sb[E:2 * E, :],
                        bass.mybir.AluOpType.mult, bass.mybir.AluOpType.add,
                    )
                    v = lg_pool.tile([E, W], f32, name="v", tag="v")
                    nc.vector.scalar_tensor_tensor(
                        v[:], b_accs[g][E:2 * E, :], LO_UNSCALE, u[:],
                        bass.mybir.AluOpType.mult, bass.mybir.AluOpType.add,
                    )
                    tk_psum = tp_pool.tile([128, 4 * E], f32, name="tk_psum")
                    for jj in range(nsub):
                        te = slice(jj * E, (jj + 1) * E)
                        tt = slice(jj * 128, (jj + 1) * 128)
                        nc.tensor.matmul(
                            tk_psum[:, te], ones_sb[:], gate_sb[:],
                            start=True, stop=False,
                        )
                        nc.tensor.matmul(
                            tk_psum[:, te], a_sb[0:E, tt], identA_sb[:],
                            start=False, stop=False,
                        )
                        nc.tensor.matmul(
                            tk_psum[:, te], v[:, tt], identB_sb[:],
                            start=False, stop=True,
                        )
                    tk_sb = lg_pool.tile([128, 4 * E], f32, name="tk_sb", tag="t")
                    nc.vector.tensor_copy(
                        tk_sb[:, 0:nsub * E], tk_psum[:, 0:nsub * E]
                    )
                    gsub = sub_base + goff[g] // 128
                    for jj in range(nsub):
                        s = gsub + jj
                        nc.vector.max(
                            vals_sb[:, s * TOP_K:(s + 1) * TOP_K],
                            tk_sb[:, jj * E:(jj + 1) * E],
                        )
                        nc.vector.max_index(
                            idx_sb[:, s * TOP_K:(s + 1) * TOP_K],
                            vals_sb[:, s * TOP_K:(s + 1) * TOP_K],
                            tk_sb[:, jj * E:(jj + 1) * E],
                        )
                # relu: clamp this pass's top-8 values (order-preserving)
                os_ = slice(sub_base * TOP_K, (sub_base + T // 128) * TOP_K)
                nc.vector.tensor_scalar_max(
                    vals_sb[:, os_], vals_sb[:, os_], 0.0
                )
                # flush this pass's outputs.  Mid-run flushes ride the idle
                # gpsimd ring (the HWDGE rings must keep streaming x and an
                # in-order trigger would stall them); the final tiny flush
                # uses the by-then-idle HWDGE rings.
                if p < N_PASS - 1:
                    nc.gpsimd.dma_start(w_out[:, os_], vals_sb[:, os_])
                    nc.gpsimd.dma_start(i_out[:, os_], idx_sb[:, os_])
                else:
                    nc.sync.dma_start(w_out[:, os_], vals_sb[:, os_])
                    nc.scalar.dma_start(i_out[:, os_], idx_sb[:, os_])
                sub_base += T // 128

    _split_multi_waits(nc)
    return nc


def _get_program():
    global _PROGRAM
    if _PROGRAM is None:
        _PROGRAM = _build_program()
    return _PROGRAM


def _make_in_maps(x, proto_k, gate):
    xf = np.ascontiguousarray(x, dtype=np.float32).reshape(TOKENS, HIDDEN)
    proto = np.asarray(proto_k, dtype=np.float32)
    ph = proto.astype(np.float16)
    pl = ((proto - ph.astype(np.float32)) * LO_SCALE).astype(np.float16)
    phpl = np.concatenate([ph.T, pl.T], axis=1)           # [4096, 128] f16
    # [row in chunk][chunk][2E]
    phpl_r = np.ascontiguousarray(
        phpl.reshape(N_CHUNK, 128, 2 * NUM_EXPERTS).transpose(1, 0, 2)
    )
    gate_row = np.ascontiguousarray(
        -np.asarray(gate, dtype=np.float32).reshape(1, NUM_EXPERTS)
    )
    toff = np.cumsum([0] + PASS_TOK)
    in_maps = []
    for cid in range(N_CORES):
        shard = xf[cid * T_CORE:(cid + 1) * T_CORE]       # [2048, 4096]
        hi = shard.astype(np.float16)
        lo = ((shard - hi.astype(np.float32)) * LO_SCALE).astype(np.float16)
        hi_t = hi.T.reshape(N_CHUNK, 128, T_CORE)         # [chunk, row, tok]
        lo_t = lo.T.reshape(N_CHUNK, 128, T_CORE)
        m = {"phpl": phpl_r, "gate_row": gate_row}
        for p, T in enumerate(PASS_TOK):
            xp = np.empty((N_CHUNK, 128, 2, T), np.float16)
            xp[:, :, 0, :] = hi_t[:, :, toff[p]:toff[p + 1]]
            xp[:, :, 1, :] = lo_t[:, :, toff[p]:toff[p + 1]]
            m[f"xp{p}"] = xp
        in_maps.append(m)
    return in_maps


def _gather(results):
    w = np.empty((TOKENS, TOP_K), np.float32)
    idx = np.empty((TOKENS, TOP_K), np.int32)
    for c in range(N_CORES):
        wo = results[c]["w_out"]                          # [128, 16*8]
        io = results[c]["i_out"].view(np.int32)
        w[c * T_CORE:(c + 1) * T_CORE] = (
            wo.reshape(128, N_SUB, TOP_K).transpose(1, 0, 2).reshape(T_CORE, TOP_K)
        )
        idx[c * T_CORE:(c + 1) * T_CORE] = (
            io.reshape(128, N_SUB, TOP_K).transpose(1, 0, 2).reshape(T_CORE, TOP_K)
        )
    return w.reshape(4, 4096, TOP_K), idx.reshape(4, 4096, TOP_K)


def run_sharded(in_maps, trace=False, trace_cores=None):
    from concourse.bass_utils import run_bass_kernel_spmd

    nc = _get_program()
    return run_bass_kernel_spmd(
        nc,
        in_maps,
        core_ids=list(range(N_CORES)),
        trace=trace,
        trace_cores=trace_cores,
    )


def kernel(x, proto_k, gate):
    in_maps = _make_in_maps(x, proto_k, gate)
    res = run_sharded(in_maps, trace=False)
    return _gather(res.results)


# revision 39
# speedup vs baseline: 1.1809x; 1.0744x over previous
"""MoE routing kernel for Trainium2 (8 NeuronCores, SPMD data-parallel).

Computes, for x [4, 4096, 4096] f32, proto_k [64, 4096] f32, gate [64] f32:
    logits = relu(x @ proto_k.T / sqrt(4096) - gate)        # [B, S, 64]
    routing_weights, selected_experts = top_k(logits, k=8)  # [B, S, 8] each

Sharding: tokens (B*S = 16384) are split evenly across 8 cores (2048 each).
proto_k / gate are replicated. No collectives needed.

Numerics: the matmul runs as a 4-term fp16 hi/lo split (x = xh + 2^-11 xl,
proto = ph + 2^-11 pl, residuals pre-scaled by 2^11 on the host so they stay
in fp16's normal range). logits = xh@ph + 2^-11 (xh@pl + xl@ph) + 2^-22
xl@pl, recombined on the DVE.  Bit-noise-level agreement with the fp32
reference (max logit perturbation ~5e-8, zero top-8 index flips) while
streaming the PE at fp16 rate.

The kernel is HBM-bandwidth bound (32 MB of x per core).  Device program:
  - x is laid out on the host as per-pass [chunk][row 128][hi|lo][token]
    fp16 blocks so every DMA reads *sequential* DRAM with 4 KB(+) contiguous
    per-partition lines; transfers are ~1.5 MB (2 chunks) on the two HWDGE
    rings (sync / scalar) running concurrently.
  - Tokens are processed in UNEVEN passes of [768, 768, 384, 128] with
    512/256-token PSUM accumulation groups.  Every pass's epilogue
    (hi/lo recombine -> relu-gate -> transpose -> top-8) overlaps the next
    pass's DMA stream; only the tiny 128-token final pass's epilogue is
    exposed after the last HBM byte.
  - Epilogue work is spread across engines: ACT (PSUM copies, relu), DVE
    (combines, Max8/MaxIndex8), PE (transposes), so the DVE never becomes
    the tail bottleneck.
  - Weights/gate/identity and the mid-run output flushes ride the gpsimd
    SWDGE ring; the final 128-token flush uses the then-idle HWDGE rings.
"""

import numpy as np

HIDDEN = 4096
NUM_EXPERTS = 64
TOP_K = 8
N_CORES = 8
TOKENS = 4 * 4096
T_CORE = TOKENS // N_CORES          # 2048 tokens per core
N_CHUNK = HIDDEN // 128             # 32 contraction chunks
N_SUB = T_CORE // 128               # 16 output sub-tiles of 128 tokens
LO_SCALE = np.float32(2.0 ** 11)
LO_UNSCALE = 2.0 ** -11

# uneven pass sizes: the last pass is tiny so the only epilogue that cannot
# overlap the DMA stream is as short as possible
PASS_TOK = [768, 768, 384, 128]
PASS_GROUPS = [[512, 256], [512, 256], [384], [128]]
# chunks per x dma_start per pass (transfer sizes ~1.5/1.5/0.75/0.5 MB:
# small enough that per-transfer latency (size / one ring's share of the
# fabric) stays a few microseconds, large enough to amortize overheads)
PASS_CHDMA = [2, 2, 2, 4]
assert sum(PASS_TOK) == T_CORE

_PROGRAM = None


def _split_multi_waits(nc):
    """walrus in this container rejects instructions carrying more sync waits
    than their ISA struct holds (setupSyncWait: 'Too many sync wait
    commands'); Drain takes one, S3_LW (matmul weight-load) ~two.  Normalize
    every instruction to a single wait by hoisting extras onto same-engine
    NOPs inserted immediately before the owner."""
    import bass_rust

    inserts = {}  # owner inst name -> list of wait-nop instructions
    for f in nc.m.functions:
        for bb in f.blocks:
            for inst in bb.instructions:
                si = inst.sync_info
                if si is None or len(si.on_wait) <= 1:
                    continue
                conds = list(si.on_wait)
                si.on_wait = conds[:1]
                eng = nc.engines[inst.engine]
                new_insts = []
                for w in conds[1:]:
                    nop = eng.nop(hint="split_wait")
                    nop.ins.sync_info = bass_rust.SyncInfo(
                        on_wait=[w], on_update=[]
                    )
                    new_insts.append(nop.ins)
                inserts[inst.name] = new_insts
    if not inserts:
        return
    # nop() appended the new instructions to whatever bb was current; strip
    # them from everywhere, then re-insert each right before its owner so
    # the engine observes every wait before executing the instruction.
    appended = {ni.name for nis in inserts.values() for ni in nis}
    for f in nc.m.functions:
        for bb in f.blocks:
            rebuilt = []
            changed = False
            for inst in bb.instructions:
                if inst.name in appended:
                    changed = True
                    continue
                if inst.name in inserts:
                    rebuilt.extend(inserts[inst.name])
                    changed = True
                rebuilt.append(inst)
            if changed:
                bb.instructions = rebuilt


def _build_program():
    import concourse.bass as bass
    import concourse.mybir as mybir
    import concourse.tile as tile

    f32 = mybir.dt.float32
    f16 = mybir.dt.float16
    u32 = mybir.dt.uint32
    E = NUM_EXPERTS
    N_PASS = len(PASS_TOK)

    nc = bass.Bass("TRN2", target_bir_lowering=False, debug=False)

    # per-pass x blocks, host-reordered to [chunk][row in chunk][hi|lo][tok]
    # fp16: fully sequential DRAM, contiguous per-partition lines.
    xp_d = [
        nc.dram_tensor(f"xp{p}", [N_CHUNK, 128, 2, PASS_TOK[p]], f16,
                       kind="ExternalInput")
        for p in range(N_PASS)
    ]
    # proto hi|lo, host-reordered to [row in chunk][chunk][2E]
    phpl_d = nc.dram_tensor("phpl", [128, N_CHUNK, 2 * E], f16,
                            kind="ExternalInput")
    gate_neg = nc.dram_tensor("gate_neg", [E, 1], f32, kind="ExternalInput")
    w_out = nc.dram_tensor("w_out", [128, N_SUB * TOP_K], f32, kind="ExternalOutput")
    i_out = nc.dram_tensor("i_out", [128, N_SUB * TOP_K], u32, kind="ExternalOutput")

    ident_dram = nc.inline_tensor(np.eye(E, dtype=np.float32), name="ident64")

    with tile.TileContext(nc) as tc:
        with (
            tc.tile_pool(name="const", bufs=1) as const_pool,
            tc.tile_pool(name="xa", bufs=12) as x_pool,
            tc.tile_pool(name="acc", bufs=6, space="PSUM") as acc_pool,
            tc.tile_pool(name="tp", bufs=2, space="PSUM") as tp_pool,
            tc.tile_pool(name="lg", bufs=3) as lg_pool,
            tc.tile_pool(name="outp", bufs=1) as out_pool,
        ):
            # --- constants (gpsimd SWDGE ring, ahead of its x share) ---
            phpl_sb = const_pool.tile([128, N_CHUNK, 2 * E], f16)
            # chunk 0 separately so the first matmul's weights land early
            nc.gpsimd.dma_start(phpl_sb[:, 0, :], phpl_d[:, 0, :])
            nc.gpsimd.dma_start(phpl_sb[:, 1:, :], phpl_d[:, 1:, :])
            gate_sb = const_pool.tile([E, 1], f32)
            nc.gpsimd.dma_start(gate_sb[:], gate_neg[:])
            ident_sb = const_pool.tile([E, E], f32)
            nc.gpsimd.dma_start(ident_sb[:], ident_dram[:])

            vals_sb = out_pool.tile([128, N_SUB * TOP_K], f32)
            idx_sb = out_pool.tile([128, N_SUB * TOP_K], u32)

            # x streams on the two HWDGE rings.  With compute running, the
            # SBUF write ports shared with the PE's ~600 GB/s of operand
            # reads cap DMA near ~400 GB/s, which two rings already reach;
            # adding the SWDGE ring as a third x path measured SLOWER
            # (Q7 emission + descriptor-ring SBUF traffic).
            rings = [nc.sync, nc.scalar]
            ring_state = [0]

            def next_ring():
                r = rings[ring_state[0] % 2]
                ring_state[0] += 1
                return r

            def emit_epilogue(p, groups, goff, a_accs, b_accs, base):
                # tk = [a_hi + 2^-11 (a_lo + b_hi) + 2^-22 b_lo]/64 - gate,
                # per 128-token subtile, transposed to [token, expert].
                # ENTIRELY on the DVE (copies, recombines, scale+gate via
                # per-partition tensor_scalar, 32x32-block transpose,
                # Max8/MaxIndex8): the in-order PE queue stays pure
                # accumulation matmuls and the sync/scalar queues stay pure
                # DMA triggers, so the epilogue can never stall the x
                # stream.  Max8 runs pre-relu (relu is monotone, so top-8
                # order is unchanged); the relu clamp lands on the [*, 8]
                # outputs at pass end.
                gorder = (reversed(range(len(groups))) if p == N_PASS - 1
                          else range(len(groups)))
                gorder = list(gorder)
                # all `a` PSUM->SBUF copies first: they free the PSUM banks
                # the NEXT pass's b-matmuls are waiting to reuse.  If they
                # queued behind a group's full DVE chain, the PE would stall
                # >3.4us at the pass boundary and the hardware activity
                # monitor would downclock it to 1.2 GHz.
                a_sbs = {}
                for g in gorder:
                    a_sb = lg_pool.tile(
                        [128, groups[g]], f32, name="a_sb", tag="a", bufs=2
                    )
                    nc.vector.tensor_copy(a_sb[:], a_accs[g][:])
                    a_sbs[g] = a_sb
                for g in gorder:
                    W = groups[g]
                    nsub = W // 128
                    a_sb = a_sbs[g]
                    u = lg_pool.tile([E, W], f32, name="u", tag="u")
                    nc.vector.scalar_tensor_tensor(
                        u[:], b_accs[g][0:E, :], 1.0, a_sb[E:2 * E, :],
                        bass.mybir.AluOpType.mult, bass.mybir.AluOpType.add,
                    )
                    v = lg_pool.tile([E, W], f32, name="v", tag="v")
                    nc.vector.scalar_tensor_tensor(
                        v[:], b_accs[g][E:2 * E, :], LO_UNSCALE, u[:],
                        bass.mybir.AluOpType.mult, bass.mybir.AluOpType.add,
                    )
                    comb = lg_pool.tile([E, W], f32, name="comb", tag="c")
                    nc.vector.scalar_tensor_tensor(
                        comb[:], v[:], LO_UNSCALE, a_sb[0:E, :],
                        bass.mybir.AluOpType.mult, bass.mybir.AluOpType.add,
                    )
                    sg = lg_pool.tile([E, W], f32, name="sg", tag="s")
                    nc.vector.tensor_scalar(
                        sg[:], comb[:], 1.0 / 64.0, gate_sb[:],
                        bass.mybir.AluOpType.mult, bass.mybir.AluOpType.add,
                    )
                    tk_psum = tp_pool.tile([128, 4 * E], f32, name="tk_psum")
                    for jj in range(nsub):
                        nc.tensor.transpose(
                            tk_psum[:, jj * E:(jj + 1) * E],
                            sg[:, jj * 128:(jj + 1) * 128],
                            ident_sb[:],
                        )
                    tk_sb = lg_pool.tile([128, 4 * E], f32, name="tk_sb", tag="t")
                    nc.vector.tensor_copy(
                        tk_sb[:, 0:nsub * E], tk_psum[:, 0:nsub * E]
                    )
                    gsub = base + goff[g] // 128
                    for jj in range(nsub):
                        s = gsub + jj
                        nc.vector.max(
                            vals_sb[:, s * TOP_K:(s + 1) * TOP_K],
                            tk_sb[:, jj * E:(jj + 1) * E],
                        )
                        nc.vector.max_index(
                            idx_sb[:, s * TOP_K:(s + 1) * TOP_K],
                            vals_sb[:, s * TOP_K:(s + 1) * TOP_K],
                            tk_sb[:, jj * E:(jj + 1) * E],
                        )
                # relu: clamp this pass's top-8 values (order-preserving)
                nsub_p = sum(groups) // 128
                os_ = slice(base * TOP_K, (base + nsub_p) * TOP_K)
                nc.vector.tensor_scalar_max(
                    vals_sb[:, os_], vals_sb[:, os_], 0.0
                )
                if p == N_PASS - 1:
                    # single output flush at the very end: every DMA ring
                    # carries x-stream triggers mid-run, and a flush trigger
                    # waiting on the epilogue would stall them (in-order
                    # queues).  By now both HWDGE rings are idle.
                    nc.sync.dma_start(w_out[:], vals_sb[:])
                    nc.scalar.dma_start(i_out[:], idx_sb[:])

            sub_base = 0  # running 128-token output subtile index
            pending = None  # deferred epilogue of the previous pass: its PE
            # matmuls must be emitted AFTER the next pass's accumulation
            # matmuls, or the in-order PE queue would stall the next pass's
            # compute (and therefore the x stream) on the epilogue's DVE
            # dependency chain.
            for p in range(N_PASS):
                T = PASS_TOK[p]
                groups = PASS_GROUPS[p]
                goff = np.cumsum([0] + groups)[:-1]
                a_accs = [
                    acc_pool.tile([128, w], f32, name=f"a_p{p}g{g}", tag="acc")
                    for g, w in enumerate(groups)
                ]
                b_accs = [
                    acc_pool.tile([128, w], f32, name=f"b_p{p}g{g}", tag="acc")
                    for g, w in enumerate(groups)
                ]

                # ---- x stream for this pass ----
                c = 0
                slot_of = {}
                last = N_PASS - 1
                while c < N_CHUNK:
                    if p == 0 and c == 0:
                        # split the very first chunk by stream and half so
                        # the first matmul waits on ~128 KB, not 1.5 MB
                        x_t = x_pool.tile([128, 1, 2, T], f16, name="x_h", tag="xt")
                        src = xp_d[p][0:1].rearrange("c r s t -> r c s t")
                        h = groups[0]
                        nc.sync.dma_start(x_t[:, 0, 0, 0:h], src[:, 0, 0, 0:h])
                        nc.scalar.dma_start(x_t[:, 0, 1, 0:h], src[:, 0, 1, 0:h])
                        nc.sync.dma_start(x_t[:, 0, 0, h:], src[:, 0, 0, h:])
                        nc.scalar.dma_start(x_t[:, 0, 1, h:], src[:, 0, 1, h:])
                        slot_of[0] = (x_t, 0)
                        c += 1
                    elif p == last and c == N_CHUNK - 2:
                        # final two chunks: one small transfer per HWDGE ring
                        # (lowest latency) so the last bytes land soon
                        x_t = x_pool.tile([128, 2, 2, T], f16, name="x_z", tag="xt")
                        src = xp_d[p][c:c + 2].rearrange("c r s t -> r c s t")
                        nc.sync.dma_start(x_t[:, 0], src[:, 0])
                        nc.scalar.dma_start(x_t[:, 1], src[:, 1])
                        slot_of[c] = (x_t, 0)
                        slot_of[c + 1] = (x_t, 1)
                        c += 2
                    else:
                        n = min(PASS_CHDMA[p], N_CHUNK - c)
                        if p == 0 and c <= 4:
                            n = 1  # small first transfers: fast PE ramp
                        if p == last:
                            n = min(n, N_CHUNK - 2 - c)
                        x_t = x_pool.tile([128, n, 2, T], f16, name="x_t", tag="xt")
                        src = xp_d[p][c:c + n].rearrange("c r s t -> r c s t")
                        next_ring().dma_start(x_t[:], src)
                        for j in range(n):
                            slot_of[c + j] = (x_t, j)
                        c += n

                # ---- accumulation matmuls ----
                # The previous pass's epilogue (which contains PE matmuls
                # gated on its DVE recombines) is emitted a few chunks INTO
                # this pass's matmul stream: late enough that its DVE inputs
                # are ready when the in-order PE reaches it, early enough
                # that its downstream DVE work isn't stuck behind this whole
                # pass (the PE absorbs the extra matmuls with its slack).
                for c in range(N_CHUNK):
                    x_t, j = slot_of[c]
                    first, lastc = (c == 0), (c == N_CHUNK - 1)
                    grange = (
                        reversed(range(len(groups)))
                        if (lastc and p == N_PASS - 1)
                        else range(len(groups))
                    )
                    for g in grange:
                        ts = slice(goff[g], goff[g] + groups[g])
                        nc.tensor.matmul(
                            a_accs[g][:], phpl_sb[:, c, :], x_t[:, j, 0, ts],
                            start=first, stop=lastc,
                        )
                        nc.tensor.matmul(
                            b_accs[g][:], phpl_sb[:, c, :], x_t[:, j, 1, ts],
                            start=first, stop=lastc,
                        )
                    if c == 6 and pending is not None:
                        emit_epilogue(*pending)
                        pending = None

                pending = (p, groups, goff, a_accs, b_accs, sub_base)
                sub_base += T // 128
            emit_epilogue(*pending)

    _split_multi_waits(nc)
    return nc


def _get_program():
    global _PROGRAM
    if _PROGRAM is None:
        _PROGRAM = _build_program()
    return _PROGRAM


def _make_in_maps(x, proto_k, gate):
    xf = np.ascontiguousarray(x, dtype=np.float32).reshape(TOKENS, HIDDEN)
    proto = np.asarray(proto_k, dtype=np.float32)
    ph = proto.astype(np.float16)
    pl = ((proto - ph.astype(np.float32)) * LO_SCALE).astype(np.float16)
    phpl = np.concatenate([ph.T, pl.T], axis=1)           # [4096, 128] f16
    # [row in chunk][chunk][2E]
    phpl_r = np.ascontiguousarray(
        phpl.reshape(N_CHUNK, 128, 2 * NUM_EXPERTS).transpose(1, 0, 2)
    )
    gate_neg = np.ascontiguousarray(
        -np.asarray(gate, dtype=np.float32).reshape(NUM_EXPERTS, 1)
    )
    toff = np.cumsum([0] + PASS_TOK)
    in_maps = []
    for cid in range(N_CORES):
        shard = xf[cid * T_CORE:(cid + 1) * T_CORE]       # [2048, 4096]
        hi = shard.astype(np.float16)
        lo = ((shard - hi.astype(np.float32)) * LO_SCALE).astype(np.float16)
        hi_t = hi.T.reshape(N_CHUNK, 128, T_CORE)         # [chunk, row, tok]
        lo_t = lo.T.reshape(N_CHUNK, 128, T_CORE)
        m = {"phpl": phpl_r, "gate_neg": gate_neg}
        for p, T in enumerate(PASS_TOK):
            xp = np.empty((N_CHUNK, 128, 2, T), np.float16)
            xp[:, :, 0, :] = hi_t[:, :, toff[p]:toff[p + 1]]
            xp[:, :, 1, :] = lo_t[:, :, toff[p]:toff[p + 1]]
            m[f"xp{p}"] = xp
        in_maps.append(m)
    return in_maps


def _gather(results):
    w = np.empty((TOKENS, TOP_K), np.float32)
    idx = np.empty((TOKENS, TOP_K), np.int32)
    for c in range(N_CORES):
        wo = results[c]["w_out"]                          # [128, 16*8]
        io = results[c]["i_out"].view(np.int32)
        w[c * T_CORE:(c + 1) * T_CORE] = (
            wo.reshape(128, N_SUB, TOP_K).transpose(1, 0, 2).reshape(T_CORE, TOP_K)
        )
        idx[c * T_CORE:(c + 1) * T_CORE] = (
            io.reshape(128, N_SUB, TOP_K).transpose(1, 0, 2).reshape(T_CORE, TOP_K)
        )
    return w.reshape(4, 4096, TOP_K), idx.reshape(4, 4096, TOP_K)


def run_sharded(in_maps, trace=False, trace_cores=None):
    from concourse.bass_utils import run_bass_kernel_spmd

    nc = _get_program()
    return run_bass_kernel_spmd(
        nc,
        in_maps,
        core_ids=list(range(N_CORES)),
        trace=trace,
        trace_cores=trace_cores,
    )


def kernel(x, proto_k, gate):
    in_maps = _make_in_maps(x, proto_k, gate)
    res = run_sharded(in_maps, trace=False)
    return _gather(res.results)
